# revision 50
# baseline (speedup 1.0000x reference)
"""Multi-head attention (b=1, n=2048, d_model=1024, 16 heads x 64) on 8 TRN2
NeuronCores, head-parallel tensor parallelism: each core computes 2 heads end
to end; the 8 partial f16 outputs (rank-128 slices of the out-proj
contraction) are summed on the host along with b_out.

The kernel is paced by the scalar engine's exp stream (64 softmax tiles of
[128,1024] at ~1.1us each), so the schedule keeps that engine saturated:
  - x arrives host-side pre-transposed/bf16 in per-chunk-contiguous layout
    (8KB DMA lines, two queues), so the device does no transposes and half
    the x DMA of an f32 feed
  - phase A: per 512-column group, q/k projections (d-blocks interleaved so
    each chunk-0 DMA arrival unlocks work) followed immediately by every
    score tile S^T = K Q^T -> exp -> P^T whose inputs exist (ramping quota),
    then v projections (+ ones columns for softmax row sums) with more
    scores; chunk-0's AV accumulation runs inside window 0
  - attention windows: AV(ci) j-steps with held-back chunk-3 score tiles
    emitted at window starts, where the PE would otherwise idle while the
    previous chunk's softmax-normalization chain frees the AV psum ring
  - softmax norm: denominator row (ones-column trick) -> DVE
    reciprocal_approx_fast -> gpsimd partition_broadcast -> DVE multiply
  - the last chunk's AV runs as two 256-wide halves so norm+out-proj of
    half A overlap the accumulation of half B, shrinking the serial tail
  - pt (probability) tiles sit at the lowest SBUF addresses: ACT writes to
    high SBUF addresses measure ~20% slower (1337ns vs 1113ns per tile)
  - out-proj f32 psum is cast to f16 on the DVE (early chunks) or the scalar
    engine (late chunks, once the exp stream has drained)
"""

import os
import sys

sys.path.insert(0, "/opt/trn_rl_repo")

import numpy as np
import ml_dtypes

import concourse.bass as bass
import concourse.tile as tile
from concourse import bacc, mybir
from concourse.bass_utils import run_bass_kernel_spmd

F32 = mybir.dt.float32
F16 = mybir.dt.float16
BF16 = mybir.dt.bfloat16

N = 2048          # sequence length
D = 1024          # d_model
H_PER_CORE = 2    # heads per core
DH = 64           # head dim
C = H_PER_CORE * DH   # per-core qkv width = 128
N_CORES = 8
P = 128
D_TILES = D // P      # 8
ICH = 512             # query-chunk width
NCH = N // ICH        # 4 chunks
NJT = N // P          # 16 j tiles

_CACHE = {}


def build_graph():
    nc = bacc.Bacc()

    # host-prepared xT in per-chunk-contiguous layout [p, chunk, o, i]
    xd_ext = nc.declare_dram_parameter("xd", [P, NCH, D_TILES, ICH], BF16,
                                       isOutput=False)
    wq_ext = nc.declare_dram_parameter("wq", [P, D_TILES, C], BF16, isOutput=False)
    wk_ext = nc.declare_dram_parameter("wk", [P, D_TILES, C], BF16, isOutput=False)
    wv_ext = nc.declare_dram_parameter("wv", [P, D_TILES, C], BF16, isOutput=False)
    wo_ext = nc.declare_dram_parameter("wo", [C, D], BF16, isOutput=False)
    bq_ext = nc.declare_dram_parameter("bq", [1, C], BF16, isOutput=False)
    bk_ext = nc.declare_dram_parameter("bk", [1, C], BF16, isOutput=False)
    bv_ext = nc.declare_dram_parameter("bv", [1, C], BF16, isOutput=False)
    out_ext = nc.declare_dram_parameter("out", [N, D], F16, isOutput=True)

    with tile.TileContext(nc) as tc:
        with (
            # pt first: low SBUF addresses make ACT writes ~20% faster
            tc.tile_pool(name="pt", bufs=52) as ptpool,
            tc.tile_pool(name="persist", bufs=1) as persist,
            tc.tile_pool(name="small", bufs=1) as small,
            tc.tile_pool(name="outsb", bufs=4) as outsb,
            tc.tile_pool(name="ps", bufs=2, space="PSUM") as ps,
        ):
            xT = persist.tile([P, D_TILES, N], BF16)
            wq_sb = persist.tile([P, D_TILES, C], BF16)
            wk_sb = persist.tile([P, D_TILES, C], BF16)
            wv_sb = persist.tile([P, D_TILES, C], BF16)
            wo_sb = persist.tile([C, D], BF16)
            bq_sb = persist.tile([1, C], BF16)
            bk_sb = persist.tile([1, C], BF16)
            bv_sb = persist.tile([1, C], BF16)
            # chunk 0 arrives per-d-block on alternating queues so the first
            # qk matmuls start as soon as possible
            nc.sync.dma_start(wq_sb[:], wq_ext[:])
            nc.gpsimd.dma_start(wk_sb[:], wk_ext[:])
            nc.sync.dma_start(bq_sb[:], bq_ext[:])
            nc.gpsimd.dma_start(bk_sb[:], bk_ext[:])
            nc.gpsimd.dma_start(bv_sb[:], bv_ext[:])
            nc.sync.dma_start(xT[:, 0:2, 0:ICH], xd_ext[:, 0, 0:2, :])
            nc.gpsimd.dma_start(xT[:, 2:4, 0:ICH], xd_ext[:, 0, 2:4, :])
            nc.sync.dma_start(xT[:, 4:6, 0:ICH], xd_ext[:, 0, 4:6, :])
            nc.gpsimd.dma_start(xT[:, 6:8, 0:ICH], xd_ext[:, 0, 6:8, :])
            nc.gpsimd.dma_start(wv_sb[:], wv_ext[:])
            # chunks 1-3 split half/half across the two queues
            for ci in range(1, NCH):
                cc = slice(ci * ICH, (ci + 1) * ICH)
                nc.sync.dma_start(xT[:, 0:4, cc], xd_ext[:, ci, 0:4, :])
                nc.gpsimd.dma_start(xT[:, 4:8, cc], xd_ext[:, ci, 4:8, :])
            nc.gpsimd.dma_start(wo_sb[:], wo_ext[:])

            ones_row = persist.tile([1, ICH], BF16)
            nc.gpsimd.memset(ones_row, 1.0)

            qT = persist.tile([P, N], BF16)          # both heads stacked
            kT0 = persist.tile([P, N], BF16)         # head0 rows 0:64, rest 0
            kT1 = persist.tile([P, N], BF16)         # head1 rows 64:128, rest 0
            nc.vector.memset(kT0[DH:P, :], 0.0)
            nc.vector.memset(kT1[0:DH, :], 0.0)
            v_sb = persist.tile([P, NJT, 2 * (DH + 1)], BF16)
            nc.vector.memset(v_sb[:], 1.0)  # ones cols survive the copies
            aT = persist.tile([P, N], BF16)

            dn = small.tile([1, 2 * ICH], F32)
            rinv = small.tile([1, 2 * ICH], F32)

            pts = {}

            def emit_score(cj, jt):
                sps = ps.tile([P, 2 * ICH], F32, tag="s", name=f"s_{cj}_{jt}")
                jc = slice(jt * P, (jt + 1) * P)
                cc = slice(cj * ICH, (cj + 1) * ICH)
                nc.tensor.matmul(sps[:, 0:ICH], kT0[:, jc], qT[:, cc],
                                 start=True, stop=True)
                nc.tensor.matmul(sps[:, ICH:], kT1[:, jc], qT[:, cc],
                                 start=True, stop=True)
                pt = ptpool.tile([P, 2 * ICH], BF16, tag="pt",
                                 name=f"pt_{cj}_{jt}")
                nc.scalar.activation(
                    pt[:], sps[:], mybir.ActivationFunctionType.Exp)
                pts[(cj, jt)] = pt

            def pt_slice(ci, j, h, off=0, w=ICH):
                col = h * ICH + off
                return pts[(ci, j)][:, col:col + w]

            def qk_proj(g):
                # q/k interleaved per d-block so each chunk-0 DMA arrival
                # unlocks two matmuls
                cc = slice(g * ICH, (g + 1) * ICH)
                qp = ps.tile([P, ICH], F32, tag="qk", name=f"qp{g}")
                kp = ps.tile([P, ICH], F32, tag="qk", name=f"kp{g}")
                for do in range(D_TILES):
                    nc.tensor.matmul(qp[:], wq_sb[:, do, :], xT[:, do, cc],
                                     start=(do == 0), stop=False)
                    nc.tensor.matmul(kp[:], wk_sb[:, do, :], xT[:, do, cc],
                                     start=(do == 0), stop=False)
                nc.tensor.matmul(qp[:], bq_sb[:], ones_row[:],
                                 start=False, stop=True)
                nc.tensor.matmul(kp[:], bk_sb[:], ones_row[:],
                                 start=False, stop=True)
                nc.vector.tensor_copy(out=qT[:, cc], in_=qp[:])
                nc.vector.tensor_copy(out=kT0[0:DH, cc], in_=kp[0:DH, :])
                nc.vector.tensor_copy(out=kT1[DH:P, cc], in_=kp[DH:P, :])

            def v_proj(g):
                vp = ps.tile([P, 4, P], F32, tag="vav", name=f"vp{g}")
                for t in range(4):
                    jt = 4 * g + t
                    jc = slice(jt * P, (jt + 1) * P)
                    for do in range(D_TILES):
                        nc.tensor.matmul(vp[:, t, :], xT[:, do, jc],
                                         wv_sb[:, do, :],
                                         start=(do == 0), stop=False)
                    nc.tensor.matmul(vp[:, t, :], ones_row[:, 0:P], bv_sb[:],
                                     start=False, stop=True)
                vin = vp.rearrange("p t (s u) -> p t s u", u=DH)
                vout = v_sb[:, 4 * g:4 * g + 4, :].rearrange(
                    "p t (s u) -> p t s u", u=DH + 1)[:, :, :, 0:DH]
                nc.vector.tensor_copy(out=vout, in_=vin)

            # scores pending emission, priority chunk-major
            pending = [(cj, jt) for cj in range(NCH) for jt in range(NJT)]

            def pop_scores(qmax, kmax, quota):
                got = 0
                i = 0
                while got < quota and i < len(pending):
                    cj, p = pending[i]
                    if cj <= qmax and p // 4 <= kmax:
                        pending.pop(i)
                        emit_score(cj, p)
                        got += 1
                    else:
                        i += 1

            # --- phase A: qk(g) then all currently-available scores the ACT
            # can chew before the next qk chunk; then v projections ---
            for g in range(NCH):
                qk_proj(g)
                pop_scores(g, g, (4, 6, 6, 6)[g])
            for g in range(NCH):
                pop_scores(NCH - 1, NCH - 1, 4)
                v_proj(g)

            av_tiles = {}

            def norm(ci, avps=None, coff=0, w=ICH):
                # normalize columns [ci*ICH+coff, +w) from avps (which hold
                # the w-wide accumulation)
                if avps is None:
                    avps = av_tiles[ci]
                for h in range(H_PER_CORE):
                    nc.vector.tensor_copy(
                        out=dn[:, h * ICH:h * ICH + w],
                        in_=avps[h][DH:DH + 1, 0:w])
                    nc.vector.reciprocal_approx_fast(
                        rinv[:, h * ICH:h * ICH + w],
                        dn[:, h * ICH:h * ICH + w])
                for h in range(H_PER_CORE):
                    rb = small.tile([DH, ICH], F32, tag=f"rbc{h}", bufs=2,
                                    name=f"rbc{h}_{ci}_{coff}")
                    nc.gpsimd.partition_broadcast(
                        rb[:, 0:w], rinv[0:1, h * ICH:h * ICH + w],
                        channels=DH)
                    nc.vector.tensor_tensor(
                        aT[h * DH:(h + 1) * DH,
                           ci * ICH + coff:ci * ICH + coff + w],
                        avps[h][0:DH, 0:w], rb[:, 0:w],
                        mybir.AluOpType.mult)

            def out_proj_step(ci, k):
                iblk = ci * (ICH // P) + k // 2
                nn = k % 2
                op = ps.tile([P, 512], F32, tag="qk", name=f"op{iblk}_{nn}")
                nc.tensor.matmul(
                    op[:], aT[:, iblk * P:(iblk + 1) * P],
                    wo_sb[:, nn * 512:(nn + 1) * 512],
                    start=True, stop=True)
                ob = outsb.tile([P, 512], F16, tag="ob",
                                name=f"ob{iblk}_{nn}")
                if ci >= 2:
                    # exp stream is done by now: the scalar engine does the
                    # cast so the DVE stays free for the final norm chain
                    nc.scalar.copy(out=ob[:], in_=op[:])
                else:
                    nc.vector.tensor_copy(out=ob[:], in_=op[:])
                # sync queue only: gpsimd must stay free for the norm
                # chain's partition_broadcasts at window boundaries
                nc.sync.dma_start(
                    out_ext[iblk * P:(iblk + 1) * P,
                            nn * 512:(nn + 1) * 512], ob[:])

            # --- attention windows; leftover chunk-3 scores are emitted at
            # window starts where the PE would otherwise idle on the norm
            # chain of the previous chunk (vav psum ring) ---
            for ci in range(NCH - 1):
                avps = [ps.tile([DH + 1, ICH], F32, tag="vav",
                                name=f"av{ci}h{h}")
                        for h in range(H_PER_CORE)]
                av_tiles[ci] = avps
                if ci > 0:
                    norm(ci - 1)
                    pop_scores(NCH - 1, NCH - 1, (0, 8, 3)[ci])
                for j in range(NJT):
                    for h in range(H_PER_CORE):
                        nc.tensor.matmul(
                            avps[h][:],
                            v_sb[:, j, h * (DH + 1):(h + 1) * (DH + 1)],
                            pt_slice(ci, j, h),
                            start=(j == 0), stop=(j == NJT - 1))
                    if ci == 0 and j < 6:
                        pop_scores(NCH - 1, NCH - 1, 2)
                    if ci == 2 and j in (2, 4, 6):
                        # last chunk-3 tiles: emitted early inside window 2 so
                        # their exps finish before AV(3) needs them (emitting
                        # them at window-3 start queues their matmuls behind
                        # all of window 2's AV work)
                        pop_scores(NCH - 1, NCH - 1, 1)
                    if ci > 0 and 6 <= j < 14:
                        out_proj_step(ci - 1, j - 6)
                for j in range(NJT):
                    del pts[(ci, j)]
            # last chunk in two 256-wide halves: norm+out of half A overlap
            # the AV accumulation of half B, shrinking the serial tail
            HW_ = ICH // 2
            ci = NCH - 1
            norm(ci - 1)
            pop_scores(NCH - 1, NCH - 1, len(pending))
            halves = []
            for half in range(2):
                avh = [ps.tile([DH + 1, HW_], F32, tag="vav",
                               name=f"av{ci}x{half}h{h}")
                       for h in range(H_PER_CORE)]
                halves.append(avh)
                for j in range(NJT):
                    for h in range(H_PER_CORE):
                        nc.tensor.matmul(
                            avh[h][:],
                            v_sb[:, j, h * (DH + 1):(h + 1) * (DH + 1)],
                            pt_slice(ci, j, h, off=half * HW_, w=HW_),
                            start=(j == 0), stop=(j == NJT - 1))
                    if half == 0:
                        if 6 <= j < 14:
                            out_proj_step(ci - 1, j - 6)
                    else:
                        if j == 0:
                            norm(ci, avps=halves[0], coff=0, w=HW_)
                        if 8 <= j < 12:
                            out_proj_step(ci, j - 8)
            norm(ci, avps=halves[1], coff=HW_, w=HW_)
            for k in range(4, 8):
                out_proj_step(ci, k)
            for j in range(NJT):
                del pts[(ci, j)]
    nc.compile()
    return nc


def _shard_inputs(x, W_qkv, b_qkv, W_out):
    bf = ml_dtypes.bfloat16
    x2d = np.asarray(x, dtype=np.float32).reshape(N, D)
    # xd[p, ci, o, i] = x[ci*ICH + i, o*P + p], bf16
    xd = np.ascontiguousarray(
        x2d.astype(bf).reshape(NCH, ICH, D_TILES, P).transpose(3, 0, 2, 1))
    Wr = np.asarray(W_qkv, dtype=np.float32).reshape(D, 3, 16, DH)
    br = np.asarray(b_qkv, dtype=np.float32).reshape(3, 16, DH)
    Wo = np.asarray(W_out, dtype=np.float32)
    scale = 1.0 / np.sqrt(DH)

    def wlayout(w):  # [D, C] -> [p, o, c]
        return np.ascontiguousarray(
            w.astype(bf).reshape(D_TILES, P, C).transpose(1, 0, 2))

    in_maps = []
    for c in range(N_CORES):
        hs = slice(2 * c, 2 * c + 2)
        in_maps.append({
            "xd": xd,
            "wq": wlayout(Wr[:, 0, hs, :].reshape(D, C) * scale),
            "wk": wlayout(Wr[:, 1, hs, :].reshape(D, C)),
            "wv": wlayout(Wr[:, 2, hs, :].reshape(D, C)),
            "wo": np.ascontiguousarray(Wo[c * C:(c + 1) * C, :].astype(bf)),
            "bq": np.ascontiguousarray(
                (br[0, hs, :].reshape(1, C) * scale).astype(bf)),
            "bk": np.ascontiguousarray(br[1, hs, :].reshape(1, C).astype(bf)),
            "bv": np.ascontiguousarray(br[2, hs, :].reshape(1, C).astype(bf)),
        })
    return in_maps


def _install_profile_hook():
    """Recreate the antenv.axon_hooks NTFF profile hook missing from this
    image (same ctypes ABI the axon boot script uses), and neuter the
    artifact upload which needs credentials we don't have."""
    if _CACHE.get("hook"):
        return
    import contextlib
    import ctypes
    import types

    mod = types.ModuleType("antenv.axon_hooks")
    _state = {}
    mod.set_axon_ntff_profile_hook = lambda h: _state.__setitem__("h", h)
    mod.get_axon_ntff_profile_hook = lambda: _state.get("h")
    sys.modules["antenv.axon_hooks"] = mod

    so_path = os.environ.get("PJRT_LIBRARY_PATH", "/opt/axon/libaxon_pjrt.so")
    lib = ctypes.CDLL(so_path)
    lib.axon_start_nrt_profile.argtypes = [
        ctypes.POINTER(ctypes.c_int64), ctypes.c_size_t]
    lib.axon_start_nrt_profile.restype = ctypes.c_int64
    lib.axon_stop_nrt_profile.argtypes = [ctypes.c_char_p]
    lib.axon_stop_nrt_profile.restype = ctypes.c_int64

    @contextlib.contextmanager
    def _hook(output_dir, device_ids):
        import jax
        jax.devices()
        if device_ids:
            ids = (ctypes.c_int64 * len(device_ids))(*device_ids)
            rc = lib.axon_start_nrt_profile(ids, len(device_ids))
        else:
            rc = lib.axon_start_nrt_profile(None, 0)
        if rc != 0:
            raise RuntimeError(f"axon_start_nrt_profile rc={rc}")
        try:
            yield
        finally:
            n = lib.axon_stop_nrt_profile(str(output_dir).encode())
            print(f"profile: {n} file(s) written to {output_dir}")

    mod.set_axon_ntff_profile_hook(_hook)

    from concourse import bass_utils as bu
    bu.upload_artifacts = lambda tmpdir: str(tmpdir)
    _CACHE["hook"] = True


def run(inputs, trace=False):
    if trace:
        _install_profile_hook()
    if "nc" not in _CACHE:
        _CACHE["nc"] = build_graph()
    nc = _CACHE["nc"]
    in_maps = _shard_inputs(
        inputs["x"], inputs["W_qkv"], inputs["b_qkv"], inputs["W_out"])
    res = run_bass_kernel_spmd(nc, in_maps, list(range(N_CORES)), trace=trace)
    acc = np.zeros((N, D), dtype=np.float32)
    for m in res.results:
        acc += np.asarray(m["out"], dtype=np.float32)
    acc += np.asarray(inputs["b_out"], dtype=np.float32)[None, :]
    return acc.reshape(1, N, D), res


def kernel(**inputs):
    out, _ = run(inputs, trace=False)
    return out


# revision 53
# speedup vs baseline: 1.0182x; 1.0182x over previous
"""Multi-head attention (b=1, n=2048, d_model=1024, 16 heads x 64) on 8 TRN2
NeuronCores, head-parallel tensor parallelism: each core computes 2 heads end
to end; the 8 partial f16 outputs (rank-128 slices of the out-proj
contraction) are summed on the host along with b_out.

The kernel is paced by the scalar engine's exp stream (64 softmax tiles of
[128,1024] at ~1.1us each), so the schedule keeps that engine saturated:
  - x arrives host-side pre-transposed/bf16 in per-chunk-contiguous layout
    (8KB DMA lines, two queues), so the device does no transposes and half
    the x DMA of an f32 feed
  - phase A: per 512-column group, q/k projections (d-blocks interleaved so
    each chunk-0 DMA arrival unlocks work) followed immediately by every
    score tile S^T = K Q^T -> exp -> P^T whose inputs exist (ramping quota),
    then v projections (+ ones columns for softmax row sums) with more
    scores; chunk-0's AV accumulation runs inside window 0
  - attention windows: AV(ci) j-steps with held-back chunk-3 score tiles
    emitted at window starts, where the PE would otherwise idle while the
    previous chunk's softmax-normalization chain frees the AV psum ring
  - softmax norm: denominator row (ones-column trick) -> DVE
    reciprocal_approx_fast -> gpsimd partition_broadcast -> DVE multiply
  - the last chunk's AV runs as two 256-wide halves so norm+out-proj of
    half A overlap the accumulation of half B, shrinking the serial tail
  - pt (probability) tiles sit at the lowest SBUF addresses: ACT writes to
    high SBUF addresses measure ~20% slower (1337ns vs 1113ns per tile)
  - out-proj f32 psum is cast to f16 on the DVE (early chunks) or the scalar
    engine (late chunks, once the exp stream has drained)
"""

import os
import sys

sys.path.insert(0, "/opt/trn_rl_repo")

import numpy as np
import ml_dtypes

import concourse.bass as bass
import concourse.tile as tile
from concourse import bacc, mybir
from concourse.bass_utils import run_bass_kernel_spmd

F32 = mybir.dt.float32
F16 = mybir.dt.float16
BF16 = mybir.dt.bfloat16

N = 2048          # sequence length
D = 1024          # d_model
H_PER_CORE = 2    # heads per core
DH = 64           # head dim
C = H_PER_CORE * DH   # per-core qkv width = 128
N_CORES = 8
P = 128
D_TILES = D // P      # 8
ICH = 512             # query-chunk width
NCH = N // ICH        # 4 chunks
NJT = N // P          # 16 j tiles

_CACHE = {}


def build_graph():
    nc = bacc.Bacc()

    # host-prepared xT in per-chunk-contiguous layout [p, chunk, o, i]
    xd_ext = nc.declare_dram_parameter("xd", [P, NCH, D_TILES, ICH], BF16,
                                       isOutput=False)
    wq_ext = nc.declare_dram_parameter("wq", [P, D_TILES, C], BF16, isOutput=False)
    wk_ext = nc.declare_dram_parameter("wk", [P, D_TILES, C], BF16, isOutput=False)
    wv_ext = nc.declare_dram_parameter("wv", [P, D_TILES, C], BF16, isOutput=False)
    wo_ext = nc.declare_dram_parameter("wo", [C, D], BF16, isOutput=False)
    bq_ext = nc.declare_dram_parameter("bq", [1, C], BF16, isOutput=False)
    bk_ext = nc.declare_dram_parameter("bk", [1, C], BF16, isOutput=False)
    bv_ext = nc.declare_dram_parameter("bv", [1, C], BF16, isOutput=False)
    out_ext = nc.declare_dram_parameter("out", [N, D], F16, isOutput=True)

    with tile.TileContext(nc) as tc:
        with (
            # pt first: low SBUF addresses make ACT writes ~20% faster
            tc.tile_pool(name="pt", bufs=52) as ptpool,
            tc.tile_pool(name="persist", bufs=1) as persist,
            tc.tile_pool(name="small", bufs=1) as small,
            tc.tile_pool(name="outsb", bufs=4) as outsb,
            tc.tile_pool(name="ps", bufs=2, space="PSUM") as ps,
        ):
            xT = persist.tile([P, D_TILES, N], BF16)
            wq_sb = persist.tile([P, D_TILES, C], BF16)
            wk_sb = persist.tile([P, D_TILES, C], BF16)
            wv_sb = persist.tile([P, D_TILES, C], BF16)
            wo_sb = persist.tile([C, D], BF16)
            bq_sb = persist.tile([1, C], BF16)
            bk_sb = persist.tile([1, C], BF16)
            bv_sb = persist.tile([1, C], BF16)
            # chunk 0 arrives per-d-block on alternating queues so the first
            # qk matmuls start as soon as possible
            nc.sync.dma_start(wq_sb[:], wq_ext[:])
            nc.gpsimd.dma_start(wk_sb[:], wk_ext[:])
            nc.sync.dma_start(bq_sb[:], bq_ext[:])
            nc.gpsimd.dma_start(bk_sb[:], bk_ext[:])
            nc.gpsimd.dma_start(bv_sb[:], bv_ext[:])
            nc.sync.dma_start(xT[:, 0:2, 0:ICH], xd_ext[:, 0, 0:2, :])
            nc.gpsimd.dma_start(xT[:, 2:4, 0:ICH], xd_ext[:, 0, 2:4, :])
            nc.sync.dma_start(xT[:, 4:6, 0:ICH], xd_ext[:, 0, 4:6, :])
            nc.gpsimd.dma_start(xT[:, 6:8, 0:ICH], xd_ext[:, 0, 6:8, :])
            nc.gpsimd.dma_start(wv_sb[:], wv_ext[:])
            # chunks 1-3 split half/half across the two queues
            for ci in range(1, NCH):
                cc = slice(ci * ICH, (ci + 1) * ICH)
                nc.sync.dma_start(xT[:, 0:4, cc], xd_ext[:, ci, 0:4, :])
                nc.gpsimd.dma_start(xT[:, 4:8, cc], xd_ext[:, ci, 4:8, :])
            nc.gpsimd.dma_start(wo_sb[:], wo_ext[:])

            ones_row = persist.tile([1, ICH], BF16)
            nc.gpsimd.memset(ones_row, 1.0)

            qT = persist.tile([P, N], BF16)          # both heads stacked
            kT0 = persist.tile([P, N], BF16)         # head0 rows 0:64, rest 0
            kT1 = persist.tile([P, N], BF16)         # head1 rows 64:128, rest 0
            nc.vector.memset(kT0[DH:P, :], 0.0)
            nc.vector.memset(kT1[0:DH, :], 0.0)
            v_sb = persist.tile([P, NJT, 2 * (DH + 1)], BF16)
            nc.vector.memset(v_sb[:], 1.0)  # ones cols survive the copies
            aT = persist.tile([P, N], BF16)

            dn = small.tile([1, 2 * ICH], F32)
            rinv = small.tile([1, 2 * ICH], F32)

            pts = {}

            def emit_score(cj, jt):
                sps = ps.tile([P, 2 * ICH], F32, tag="s", name=f"s_{cj}_{jt}")
                jc = slice(jt * P, (jt + 1) * P)
                cc = slice(cj * ICH, (cj + 1) * ICH)
                nc.tensor.matmul(sps[:, 0:ICH], kT0[:, jc], qT[:, cc],
                                 start=True, stop=True)
                nc.tensor.matmul(sps[:, ICH:], kT1[:, jc], qT[:, cc],
                                 start=True, stop=True)
                pt = ptpool.tile([P, 2 * ICH], BF16, tag="pt",
                                 name=f"pt_{cj}_{jt}")
                nc.scalar.activation(
                    pt[:], sps[:], mybir.ActivationFunctionType.Exp)
                pts[(cj, jt)] = pt

            def pt_slice(ci, j, h, off=0, w=ICH):
                col = h * ICH + off
                return pts[(ci, j)][:, col:col + w]

            def qk_proj(g):
                # q/k interleaved per d-block so each chunk-0 DMA arrival
                # unlocks two matmuls
                cc = slice(g * ICH, (g + 1) * ICH)
                qp = ps.tile([P, ICH], F32, tag="qk", name=f"qp{g}")
                kp = ps.tile([P, ICH], F32, tag="qk", name=f"kp{g}")
                for do in range(D_TILES):
                    nc.tensor.matmul(qp[:], wq_sb[:, do, :], xT[:, do, cc],
                                     start=(do == 0), stop=False)
                    nc.tensor.matmul(kp[:], wk_sb[:, do, :], xT[:, do, cc],
                                     start=(do == 0), stop=False)
                nc.tensor.matmul(qp[:], bq_sb[:], ones_row[:],
                                 start=False, stop=True)
                nc.tensor.matmul(kp[:], bk_sb[:], ones_row[:],
                                 start=False, stop=True)
                nc.vector.tensor_copy(out=qT[:, cc], in_=qp[:])
                nc.vector.tensor_copy(out=kT0[0:DH, cc], in_=kp[0:DH, :])
                nc.vector.tensor_copy(out=kT1[DH:P, cc], in_=kp[DH:P, :])

            def v_proj(g):
                vp = ps.tile([P, 4, P], F32, tag="vav", name=f"vp{g}")
                for t in range(4):
                    jt = 4 * g + t
                    jc = slice(jt * P, (jt + 1) * P)
                    for do in range(D_TILES):
                        nc.tensor.matmul(vp[:, t, :], xT[:, do, jc],
                                         wv_sb[:, do, :],
                                         start=(do == 0), stop=False)
                    nc.tensor.matmul(vp[:, t, :], ones_row[:, 0:P], bv_sb[:],
                                     start=False, stop=True)
                vin = vp.rearrange("p t (s u) -> p t s u", u=DH)
                vout = v_sb[:, 4 * g:4 * g + 4, :].rearrange(
                    "p t (s u) -> p t s u", u=DH + 1)[:, :, :, 0:DH]
                nc.vector.tensor_copy(out=vout, in_=vin)

            # scores pending emission, priority chunk-major
            pending = [(cj, jt) for cj in range(NCH) for jt in range(NJT)]

            def pop_scores(qmax, kmax, quota):
                got = 0
                i = 0
                while got < quota and i < len(pending):
                    cj, p = pending[i]
                    if cj <= qmax and p // 4 <= kmax:
                        pending.pop(i)
                        emit_score(cj, p)
                        got += 1
                    else:
                        i += 1

            # --- phase A: qk(g) then all currently-available scores the ACT
            # can chew before the next qk chunk; then v projections ---
            for g in range(NCH):
                qk_proj(g)
                pop_scores(g, g, (4, 6, 6, 6)[g])
            for g in range(NCH):
                pop_scores(NCH - 1, NCH - 1, 4)
                v_proj(g)

            av_tiles = {}

            def norm(ci, avps=None, coff=0, w=ICH):
                # normalize columns [ci*ICH+coff, +w) from avps (which hold
                # the w-wide accumulation)
                if avps is None:
                    avps = av_tiles[ci]
                for h in range(H_PER_CORE):
                    nc.vector.tensor_copy(
                        out=dn[:, h * ICH:h * ICH + w],
                        in_=avps[h][DH:DH + 1, 0:w])
                    nc.vector.reciprocal_approx_fast(
                        rinv[:, h * ICH:h * ICH + w],
                        dn[:, h * ICH:h * ICH + w])
                for h in range(H_PER_CORE):
                    rb = small.tile([DH, ICH], F32, tag=f"rbc{h}", bufs=2,
                                    name=f"rbc{h}_{ci}_{coff}")
                    nc.gpsimd.partition_broadcast(
                        rb[:, 0:w], rinv[0:1, h * ICH:h * ICH + w],
                        channels=DH)
                    nc.vector.tensor_tensor(
                        aT[h * DH:(h + 1) * DH,
                           ci * ICH + coff:ci * ICH + coff + w],
                        avps[h][0:DH, 0:w], rb[:, 0:w],
                        mybir.AluOpType.mult)

            def out_proj_step(ci, k):
                iblk = ci * (ICH // P) + k // 2
                nn = k % 2
                op = ps.tile([P, 512], F32, tag="qk", name=f"op{iblk}_{nn}")
                nc.tensor.matmul(
                    op[:], aT[:, iblk * P:(iblk + 1) * P],
                    wo_sb[:, nn * 512:(nn + 1) * 512],
                    start=True, stop=True)
                ob = outsb.tile([P, 512], F16, tag="ob",
                                name=f"ob{iblk}_{nn}")
                if ci >= 2:
                    # exp stream is done by now: the scalar engine does the
                    # cast so the DVE stays free for the final norm chain
                    nc.scalar.copy(out=ob[:], in_=op[:])
                else:
                    nc.vector.tensor_copy(out=ob[:], in_=op[:])
                # sync queue only: gpsimd must stay free for the norm
                # chain's partition_broadcasts at window boundaries
                nc.sync.dma_start(
                    out_ext[iblk * P:(iblk + 1) * P,
                            nn * 512:(nn + 1) * 512], ob[:])

            # --- attention windows; leftover chunk-3 scores are emitted at
            # window starts where the PE would otherwise idle on the norm
            # chain of the previous chunk (vav psum ring) ---
            for ci in range(NCH - 1):
                avps = [ps.tile([DH + 1, ICH], F32, tag="vav",
                                name=f"av{ci}h{h}")
                        for h in range(H_PER_CORE)]
                av_tiles[ci] = avps
                if ci > 0:
                    norm(ci - 1)
                    pop_scores(NCH - 1, NCH - 1, (0, 8, 3)[ci])
                for j in range(NJT):
                    for h in range(H_PER_CORE):
                        nc.tensor.matmul(
                            avps[h][:],
                            v_sb[:, j, h * (DH + 1):(h + 1) * (DH + 1)],
                            pt_slice(ci, j, h),
                            start=(j == 0), stop=(j == NJT - 1))
                    if ci == 0 and j < 6:
                        pop_scores(NCH - 1, NCH - 1, 2)
                    if ci == 2 and j in (2, 4):
                        # late chunk-3 tiles: emitted early inside window 2 so
                        # their exps finish before AV(3) needs them
                        pop_scores(NCH - 1, NCH - 1, 1)
                    if ci > 0 and 6 <= j < (12 if ci == 2 else 14):
                        out_proj_step(ci - 1, j - 6)
                if ci == 2:
                    # trailing out-proj steps fill the PE while the norm(2)
                    # chain frees the AV psum ring for window 3
                    out_proj_step(1, 6)
                    out_proj_step(1, 7)
                for j in range(NJT):
                    del pts[(ci, j)]
            # last chunk in two 256-wide halves: norm+out of half A overlap
            # the AV accumulation of half B, shrinking the serial tail
            HW_ = ICH // 2
            ci = NCH - 1
            norm(ci - 1)
            pop_scores(NCH - 1, NCH - 1, len(pending))
            halves = []
            for half in range(2):
                avh = [ps.tile([DH + 1, HW_], F32, tag="vav",
                               name=f"av{ci}x{half}h{h}")
                       for h in range(H_PER_CORE)]
                halves.append(avh)
                for j in range(NJT):
                    for h in range(H_PER_CORE):
                        nc.tensor.matmul(
                            avh[h][:],
                            v_sb[:, j, h * (DH + 1):(h + 1) * (DH + 1)],
                            pt_slice(ci, j, h, off=half * HW_, w=HW_),
                            start=(j == 0), stop=(j == NJT - 1))
                    if half == 0:
                        if 6 <= j < 14:
                            out_proj_step(ci - 1, j - 6)
                    else:
                        if j == 0:
                            norm(ci, avps=halves[0], coff=0, w=HW_)
                        if 8 <= j < 12:
                            out_proj_step(ci, j - 8)
            norm(ci, avps=halves[1], coff=HW_, w=HW_)
            for k in range(4, 8):
                out_proj_step(ci, k)
            for j in range(NJT):
                del pts[(ci, j)]
    nc.compile()
    return nc


def _shard_inputs(x, W_qkv, b_qkv, W_out):
    bf = ml_dtypes.bfloat16
    x2d = np.asarray(x, dtype=np.float32).reshape(N, D)
    # xd[p, ci, o, i] = x[ci*ICH + i, o*P + p], bf16
    xd = np.ascontiguousarray(
        x2d.astype(bf).reshape(NCH, ICH, D_TILES, P).transpose(3, 0, 2, 1))
    Wr = np.asarray(W_qkv, dtype=np.float32).reshape(D, 3, 16, DH)
    br = np.asarray(b_qkv, dtype=np.float32).reshape(3, 16, DH)
    Wo = np.asarray(W_out, dtype=np.float32)
    scale = 1.0 / np.sqrt(DH)

    def wlayout(w):  # [D, C] -> [p, o, c]
        return np.ascontiguousarray(
            w.astype(bf).reshape(D_TILES, P, C).transpose(1, 0, 2))

    in_maps = []
    for c in range(N_CORES):
        hs = slice(2 * c, 2 * c + 2)
        in_maps.append({
            "xd": xd,
            "wq": wlayout(Wr[:, 0, hs, :].reshape(D, C) * scale),
            "wk": wlayout(Wr[:, 1, hs, :].reshape(D, C)),
            "wv": wlayout(Wr[:, 2, hs, :].reshape(D, C)),
            "wo": np.ascontiguousarray(Wo[c * C:(c + 1) * C, :].astype(bf)),
            "bq": np.ascontiguousarray(
                (br[0, hs, :].reshape(1, C) * scale).astype(bf)),
            "bk": np.ascontiguousarray(br[1, hs, :].reshape(1, C).astype(bf)),
            "bv": np.ascontiguousarray(br[2, hs, :].reshape(1, C).astype(bf)),
        })
    return in_maps


def _install_profile_hook():
    """Recreate the antenv.axon_hooks NTFF profile hook missing from this
    image (same ctypes ABI the axon boot script uses), and neuter the
    artifact upload which needs credentials we don't have."""
    if _CACHE.get("hook"):
        return
    import contextlib
    import ctypes
    import types

    mod = types.ModuleType("antenv.axon_hooks")
    _state = {}
    mod.set_axon_ntff_profile_hook = lambda h: _state.__setitem__("h", h)
    mod.get_axon_ntff_profile_hook = lambda: _state.get("h")
    sys.modules["antenv.axon_hooks"] = mod

    so_path = os.environ.get("PJRT_LIBRARY_PATH", "/opt/axon/libaxon_pjrt.so")
    lib = ctypes.CDLL(so_path)
    lib.axon_start_nrt_profile.argtypes = [
        ctypes.POINTER(ctypes.c_int64), ctypes.c_size_t]
    lib.axon_start_nrt_profile.restype = ctypes.c_int64
    lib.axon_stop_nrt_profile.argtypes = [ctypes.c_char_p]
    lib.axon_stop_nrt_profile.restype = ctypes.c_int64

    @contextlib.contextmanager
    def _hook(output_dir, device_ids):
        import jax
        jax.devices()
        if device_ids:
            ids = (ctypes.c_int64 * len(device_ids))(*device_ids)
            rc = lib.axon_start_nrt_profile(ids, len(device_ids))
        else:
            rc = lib.axon_start_nrt_profile(None, 0)
        if rc != 0:
            raise RuntimeError(f"axon_start_nrt_profile rc={rc}")
        try:
            yield
        finally:
            n = lib.axon_stop_nrt_profile(str(output_dir).encode())
            print(f"profile: {n} file(s) written to {output_dir}")

    mod.set_axon_ntff_profile_hook(_hook)

    from concourse import bass_utils as bu
    bu.upload_artifacts = lambda tmpdir: str(tmpdir)
    _CACHE["hook"] = True


def run(inputs, trace=False):
    if trace:
        _install_profile_hook()
    if "nc" not in _CACHE:
        _CACHE["nc"] = build_graph()
    nc = _CACHE["nc"]
    in_maps = _shard_inputs(
        inputs["x"], inputs["W_qkv"], inputs["b_qkv"], inputs["W_out"])
    res = run_bass_kernel_spmd(nc, in_maps, list(range(N_CORES)), trace=trace)
    acc = np.zeros((N, D), dtype=np.float32)
    for m in res.results:
        acc += np.asarray(m["out"], dtype=np.float32)
    acc += np.asarray(inputs["b_out"], dtype=np.float32)[None, :]
    return acc.reshape(1, N, D), res


def kernel(**inputs):
    out, _ = run(inputs, trace=False)
    return out
